# revision 15
# baseline (speedup 1.0000x reference)
"""Trainium2 Bass kernel for CausalSequenceCML.

Math (reference, per step, grid g laid out (B, C, T)):
    mapped  = r * g * (1 - g)
    local   = causal depthwise conv1d(mapped, K, left pad 3)   # per channel
    physics = (1 - eps) * mapped + eps * local
    g'      = (1 - beta) * physics + beta * x0                 # x0 = initial grid
Because r, eps, beta, K are per-channel constants and the conv is linear,
the whole update is affine in sq = (g - 0.5)^2:
    g' = D - C3*sq[t] - C2*sq[t-1] - C1*sq[t-2] - C0*sq[t-3]
    Cj = (1-beta)*eps*r*K[j]  (j=0,1,2);  C3 = (1-beta)*r*((1-eps)+eps*K[3])
    D  = beta*x0 + 0.25*(C0+C1+C2+C3)
Left-boundary: conv pads with zeros => sq pad cols held at 0.25.

Sharding: C=512 split across 8 cores (64 ch each). Per core the (B=4, 64, T)
block is 256 rows = 2 SBUF tiles of (128, 4096): channels+batch on
partitions, time on the free dim. Rows are b-major so row p of both tiles is
the same channel => identical per-partition coefficient vectors.

Engine split, per step per tile (all fp32: the chaotic map amplifies
rounding ~1e4x over 16 steps, so 16-bit and float32r both fail):
 - ScalarE: sq = Square(g' - 0.5). For the DVE region it reads g' from
   SBUF; for the PE region it reads g' DIRECTLY FROM PSUM (psum-direct) --
   no PSUM->SBUF copies during the 16 steps (ScalarE also has the faster
   PSUM port). Copies happen only on the final step.
 - VectorE: columns [0, PE_SPLIT): 4 fused scalar_tensor_tensor ops
   (tap multiply-accumulate, 1 elem/cycle, D folded into op1's in1).
 - TensorE: columns [PE_SPLIT, T): per 512-col PSUM bank, 4 diagonal fp32
   matmuls W=diag(-Cj) (time shifts via the moving operand's column offset)
   + a 5th identity-diagonal matmul adding D, so PSUM holds g' directly.
   fp32 matmul = 4 cyc/col (2 half-speed passes) -- measured: shifting work
   toward PE below ~2700 cols is a net loss, PE/DVE balance is at ~2720.
GPSIMD stays idle (shares its SBUF port with DVE's 2-port ops: measured
2.8x slowdown when used). The two tiles pipeline each other: ACT squares
tile0 for step s+1 while DVE runs tile1's chain for step s.
"""

import numpy as np

from concourse import bacc, mybir
import concourse.tile as tile
from concourse.bass_utils import run_bass_kernel_spmd

B, T, C = 4, 4096, 512
N_CORES = 8
CPC = C // N_CORES          # channels per core = 64
ROWS = B * CPC              # 256 rows per core
HALVES = ROWS // 128        # 2 SBUF tiles per core
CLAMP = 1e-4
F32 = mybir.dt.float32

# Column split: DVE handles [0, PE_SPLIT), TensorE handles [PE_SPLIT, T).
# DVE/PE busy-time balance is at ~2720-2790 (DVE 8.33ns/col/step, PE
# 16.7ns/col/step at fp32's 4 cyc/col); measurements across this range are
# within device noise, slightly DVE-heavy is robust against PE HAM
# re-throttle.
PE_SPLIT = 2752
PE_BLOCK = 512
# Read next-step squares for the PE region straight from PSUM (no per-step
# PSUM->SBUF copies). Needs PE-region banks (2 tiles x ceil(region/512))
# <= 8; at PE_SPLIT=2720 that is 6.
PSUM_DIRECT = True

_compiled = {}


def _build(steps: int, reps: int = 1):
    # reps>1 (timing harness only): wrap the step body in a hardware loop
    # so one dispatch runs the 16-step compute `reps` times back-to-back;
    # differencing two rep counts isolates pure compute.
    nc = bacc.Bacc("TRN2", target_bir_lowering=False, debug=False)

    x = nc.dram_tensor("x", [ROWS, T], F32, kind="ExternalInput").ap()
    coef = nc.dram_tensor("coef", [ROWS, 6], F32, kind="ExternalInput").ap()
    out = nc.dram_tensor("out", [ROWS, T], F32, kind="ExternalOutput").ap()
    wdiag = nc.dram_tensor("wdiag", [ROWS, 640], F32, kind="ExternalInput").ap()

    x_h = x.rearrange("(h p) t -> h p t", p=128)
    out_h = out.rearrange("(h p) t -> h p t", p=128)
    coef_h = coef.rearrange("(h p) c -> h p c", p=128)
    wdiag_h = wdiag.rearrange("(h p) c -> h p c", p=128)

    mult = mybir.AluOpType.mult
    add = mybir.AluOpType.add

    pe_blocks = []
    c = PE_SPLIT
    while c < T:
        n = min(PE_BLOCK, T - c)
        pe_blocks.append((c, n))
        c += n
    assert HALVES * len(pe_blocks) <= 8 or not PSUM_DIRECT

    with tile.TileContext(nc) as tc:
        with tc.tile_pool(name="state", bufs=1) as pool, \
             tc.tile_pool(name="psum", bufs=8, space="PSUM") as pspool:
            neg_half = pool.tile([128, 1], F32, tag="neg_half", name="neg_half")
            nc.vector.memset(neg_half[:], -0.5)
            gA, gB, sq, D, cf, wd = [], [], [], [], [], []
            for h in range(HALVES):
                gA.append(pool.tile([128, T], F32, tag=f"gA{h}", name=f"gA{h}"))
                gB.append(pool.tile([128, T], F32, tag=f"gB{h}", name=f"gB{h}"))
                sq.append(pool.tile([128, T + 3], F32, tag=f"sq{h}", name=f"sq{h}"))
                D.append(pool.tile([128, T], F32, tag=f"D{h}", name=f"D{h}"))
                cf.append(pool.tile([128, 6], F32, tag=f"cf{h}", name=f"cf{h}"))
                wd.append(pool.tile([128, 640], F32, tag=f"wd{h}", name=f"wd{h}"))
                nc.sync.dma_start(out=wd[h][:], in_=wdiag_h[h])
            HX = T // 2
            for h in range(HALVES):
                nc.sync.dma_start(out=cf[h][:], in_=coef_h[h])
                # x in two column halves so step-0 squares start earlier
                nc.sync.dma_start(out=gA[h][:, 0:HX], in_=x_h[h][:, 0:HX])
                nc.sync.dma_start(out=gA[h][:, HX:T], in_=x_h[h][:, HX:T])
                # pad columns stay at sq-of-zero = 0.25 forever
                nc.vector.memset(sq[h][:, 0:3], 0.25)
            # TensorE HAM warm-up: ~7us of dummy matmuls on wd during the
            # x DMA window so step-0 matmuls run at full clock. Uses the
            # "ps" tag so PSUM stays within the 8-bank rotation.
            warm = pspool.tile([128, PE_BLOCK], F32, tag="ps", name="warm")
            for _ in range(2):
                nc.tensor.matmul(warm[:, :512], wd[0][:, 0:128],
                                 wd[0][:, 0:512], start=True, stop=True)
            for h in range(HALVES):
                # D = beta * x0 + dconst (per half, gated on its x DMA)
                for c0, c1 in ((0, HX), (HX, T)):
                    nc.vector.tensor_scalar(
                        D[h][:, c0:c1], gA[h][:, c0:c1], cf[h][:, 4:5],
                        cf[h][:, 5:6], mult, add,
                    )

            def dve_chain(h, nxt):
                # g' = (sq[t]*negC3 + D) + sq[t-1]*negC2 + sq[t-2]*negC1
                #      + sq[t-3]*negC0   -- fused mult+add per tap
                nc.vector.scalar_tensor_tensor(
                    nxt[h][:, 0:PE_SPLIT], sq[h][:, 3:3 + PE_SPLIT],
                    cf[h][:, 0:1], D[h][:, 0:PE_SPLIT], mult, add,
                )
                for j, off in ((1, 2), (2, 1), (3, 0)):
                    nc.vector.scalar_tensor_tensor(
                        nxt[h][:, 0:PE_SPLIT], sq[h][:, off:off + PE_SPLIT],
                        cf[h][:, j:j + 1], nxt[h][:, 0:PE_SPLIT], mult, add,
                    )

            def pe_mms(s, h):
                # psum = sum_k diag(-C_{3-k}) @ sq[:, off+c0:] + I @ D
                # (accumulated in-bank) => psum holds g' directly.
                res = []
                for (c0, n) in pe_blocks:
                    ps = pspool.tile([128, PE_BLOCK], F32, tag="ps",
                                     name=f"ps{s}_{h}_{c0}")
                    for k in range(4):
                        off = 3 - k
                        nc.tensor.matmul(
                            ps[:, :n], wd[h][:, k * 128:(k + 1) * 128],
                            sq[h][:, off + c0:off + c0 + n],
                            start=(k == 0), stop=False,
                        )
                    nc.tensor.matmul(
                        ps[:, :n], wd[h][:, 512:640], D[h][:, c0:c0 + n],
                        start=False, stop=True,
                    )
                    res.append((h, c0, n, ps))
                return res

            def emit_steps():
                # initial squares from gA, per x-DMA half
                for h in range(HALVES):
                    for c0, c1 in ((0, HX), (HX, T)):
                        nc.scalar.activation(
                            sq[h][:, 3 + c0:3 + c1], gA[h][:, c0:c1],
                            mybir.ActivationFunctionType.Square,
                            bias=neg_half[:],
                        )
                for s in range(steps):
                    nxt = gB if s % 2 == 0 else gA
                    last = s == steps - 1
                    step_psums = []
                    for h in range(HALVES):
                        step_psums += pe_mms(s, h)
                    for h in range(HALVES):
                        dve_chain(h, nxt)
                    if PSUM_DIRECT and not last:
                        # squares for step s+1, per tile: FIRST the SBUF
                        # (DVE-region) square -- it gates the next DVE chain
                        # and is ready as soon as tile h's chain ends -- then
                        # that tile's PSUM-region squares. Queuing any
                        # B-square (gated on late matmuls) ahead of an
                        # A-square head-of-line-blocks ACT and stalls DVE.
                        for h in range(HALVES):
                            nc.scalar.activation(
                                sq[h][:, 3:3 + PE_SPLIT],
                                nxt[h][:, 0:PE_SPLIT],
                                mybir.ActivationFunctionType.Square,
                                bias=neg_half[:],
                            )
                            for (hh, c0, n, ps) in step_psums:
                                if hh != h:
                                    continue
                                nc.scalar.activation(
                                    sq[h][:, 3 + c0:3 + c0 + n], ps[:, :n],
                                    mybir.ActivationFunctionType.Square,
                                    bias=neg_half[:],
                                )
                    else:
                        # copy psum out, then square everything from SBUF
                        for (h, c0, n, ps) in step_psums:
                            nc.scalar.copy(nxt[h][:, c0:c0 + n], ps[:, :n])
                        if not last:
                            for h in range(HALVES):
                                nc.scalar.activation(
                                    sq[h][:, 3:3 + T], nxt[h][:],
                                    mybir.ActivationFunctionType.Square,
                                    bias=neg_half[:],
                                )

            if reps == 1:
                emit_steps()
            else:
                # timing-harness only: repeat the whole step body on-device
                assert steps % 2 == 0, "reps needs even steps"
                with tc.For_i(0, reps, 1):
                    emit_steps()

            # Output: clamp+DMA in chunks so the t0 DVE-region chunk drains
            # while tile1's final chain still runs. Chunk boundary 2048 is
            # inside the DVE region, so the first chunk gates only on the
            # final DVE chain, not on the last-step PSUM copies.
            fin = gA if steps % 2 == 0 else gB
            for h in range(HALVES):
                for c0, c1 in ((0, HX), (HX, T)):
                    nc.vector.tensor_scalar(
                        fin[h][:, c0:c1], fin[h][:, c0:c1], CLAMP, 1.0 - CLAMP,
                        mybir.AluOpType.max, mybir.AluOpType.min,
                    )
                    nc.sync.dma_start(out=out_h[h][:, c0:c1],
                                      in_=fin[h][:, c0:c1])

    nc.compile()
    return nc


def get_nc(steps: int, reps: int = 1):
    key = (steps, reps, PE_SPLIT, PSUM_DIRECT)
    if key not in _compiled:
        _compiled[key] = _build(steps, reps)
    return _compiled[key]


def _host_prep(drive, r, eps, beta, K_causal):
    """Per-core input maps: x (256,T), coef (256,6), wdiag (256,640)."""
    drive = np.asarray(drive, np.float32)
    r = np.asarray(r, np.float32)
    eps = np.asarray(eps, np.float32)
    beta = np.asarray(beta, np.float32)
    K = np.asarray(K_causal, np.float32)[:, 0, :]  # (C, 4)

    one_m_b = 1.0 - beta
    C0 = one_m_b * eps * r * K[:, 0]
    C1 = one_m_b * eps * r * K[:, 1]
    C2 = one_m_b * eps * r * K[:, 2]
    C3 = one_m_b * r * ((1.0 - eps) + eps * K[:, 3])
    dconst = 0.25 * (C0 + C1 + C2 + C3)

    in_maps = []
    idx = np.arange(128)
    for i in range(N_CORES):
        sl = slice(i * CPC, (i + 1) * CPC)
        xs = np.ascontiguousarray(
            drive[:, :, sl].transpose(0, 2, 1).reshape(ROWS, T), np.float32
        )
        cs = np.stack(
            [np.tile(-C3[sl], B), np.tile(-C2[sl], B), np.tile(-C1[sl], B),
             np.tile(-C0[sl], B), np.tile(beta[sl], B), np.tile(dconst[sl], B)],
            axis=1,
        ).astype(np.float32)
        blocks = [-C3, -C2, -C1, -C0, np.ones(C, np.float32)]
        wdg = np.zeros((ROWS, 128 * len(blocks)), np.float32)
        for k, arr in enumerate(blocks):
            rows = np.tile(np.asarray(arr, np.float32)[sl], B)  # (ROWS,)
            for h in range(HALVES):
                wdg[h * 128 + idx, k * 128 + idx] = rows[h * 128 + idx]
        in_maps.append({"x": xs, "coef": np.ascontiguousarray(cs),
                        "wdiag": wdg})
    return in_maps


def kernel(drive, r, eps, beta, K_causal, steps):
    steps = int(steps)
    nc = get_nc(steps)
    in_maps = _host_prep(drive, r, eps, beta, K_causal)
    res = run_bass_kernel_spmd(nc, in_maps, list(range(N_CORES)))
    parts = [
        res.results[i]["out"].reshape(B, CPC, T).transpose(0, 2, 1)
        for i in range(N_CORES)
    ]
    return np.ascontiguousarray(np.concatenate(parts, axis=2), np.float32)
